# revision 6
# baseline (speedup 1.0000x reference)
"""Multi-head attention (EMB=512, HEADS=8, x:(4,2048,512)) on 8 Trainium2 cores.

Sharding: zero-collective split — core c handles batch c//2, query rows
(c%2)*1024..(c%2+1)*1024, ALL heads.  K/V projections for the full batch are
computed redundantly on the 2 cores sharing a batch (16% extra PE work, but no
collectives at all).

Device-side dataflow (per core, everything SBUF-resident):
  xT (host-transposed)           [512, 2048]   keys reordered so queries first
  Q^T = WqT.T @ xT  (+bq)        [512, 1024]   feature-major
  K^T = WkT.T @ xT  (+bk)        [512, 2048]   feature-major
  V~  = xT.T @ WvT  (+bv, ones)  [2048, 8*65]  token-major, per-head ones col
  S^T = K^T_h.T @ Q^T_h          [2048, 512]   per (head, query-chunk)
  P^T = exp(S^T / sqrt(512))     (ScalarE, fused drain from PSUM)
  outT~ = V~_h.T @ P^T           [65, 512]     row 64 = softmax denominator
  R = sel.T @ recip(sums)        partition-broadcast of 1/denominator via PE
  outT = outT~ * R               normalized, feature-major
  y = outT.T @ WoT (+bo)         [1024, 512]   token-major, DMA out
"""

import sys
import os

for _p in ("/opt/trn_rl_repo", "/root/.axon_site/_ro/trn_rl_repo"):
    if os.path.isdir(_p) and _p not in sys.path:
        sys.path.append(_p)

import numpy as np

EMB = 512
HEADS = 8
D = 64  # head dim
B = 4
N = 2048  # keys / tokens per batch
HALF = 1024  # queries per core
P = 128
NCORES = 8
KT4 = EMB // P  # 4 contraction tiles
SCALE = float(1.0 / np.sqrt(np.float32(EMB)))

_CACHE = {}


def _build_program(debug=False):
    from concourse import bacc
    import concourse.mybir as mybir
    import concourse.tile as tile
    from contextlib import ExitStack

    dt = mybir.dt.float32
    nc = bacc.Bacc("TRN2", target_bir_lowering=False)

    xT_d = nc.dram_tensor("xT", [KT4, P, N], dt, kind="ExternalInput")
    wq_d = nc.dram_tensor("wq", [KT4, P, EMB], dt, kind="ExternalInput")
    wk_d = nc.dram_tensor("wk", [KT4, P, EMB], dt, kind="ExternalInput")
    wv_d = nc.dram_tensor("wv", [KT4, P, EMB], dt, kind="ExternalInput")
    wo_d = nc.dram_tensor("wo", [KT4, P, EMB], dt, kind="ExternalInput")
    bq_d = nc.dram_tensor("bq2", [P, KT4], dt, kind="ExternalInput")
    bk_d = nc.dram_tensor("bk2", [P, KT4], dt, kind="ExternalInput")
    bvr_d = nc.dram_tensor("bvr", [P, EMB], dt, kind="ExternalInput")
    bor_d = nc.dram_tensor("bor", [P, EMB], dt, kind="ExternalInput")
    sel_d = nc.dram_tensor("sel", [HEADS, KT4, P], dt, kind="ExternalInput")
    y_d = nc.dram_tensor("y", [HALF, EMB], dt, kind="ExternalOutput")
    if debug:
        dQT = nc.dram_tensor("dQT", [P, KT4, HALF], dt, kind="ExternalOutput")
        dKT = nc.dram_tensor("dKT", [P, KT4, N], dt, kind="ExternalOutput")
        dVb = nc.dram_tensor("dVb", [P, 16, HEADS, D + 8], dt, kind="ExternalOutput")
        dsums = nc.dram_tensor("dsums", [HEADS, HALF], dt, kind="ExternalOutput")
        doutT = nc.dram_tensor("doutT", [P, KT4, HALF], dt, kind="ExternalOutput")
        dPT = nc.dram_tensor("dPT", [P, 1024], dt, kind="ExternalOutput")
        drsum = nc.dram_tensor("drsum", [HEADS, HALF], dt, kind="ExternalOutput")
        doutU = nc.dram_tensor("doutU", [P, KT4, HALF], dt, kind="ExternalOutput")

    Exp = mybir.ActivationFunctionType.Exp
    mult = mybir.AluOpType.mult
    add = mybir.AluOpType.add

    with tile.TileContext(nc) as tc, ExitStack() as ctx:
        # "big" slots (8KB/part): 4 x xT during projections, then recycled as
        # P^T chunks during attention.
        big = ctx.enter_context(tc.tile_pool(name="big", bufs=4))
        wp = ctx.enter_context(tc.tile_pool(name="wp", bufs=1))
        pers = ctx.enter_context(tc.tile_pool(name="pers", bufs=1))
        yp = ctx.enter_context(tc.tile_pool(name="yp", bufs=2))
        # PSUM: tag "s" 3 x [128,1024] slots (6 banks) + tag "pv" 2 x 1 bank
        ps = ctx.enter_context(tc.tile_pool(name="ps", bufs=3, space="PSUM"))

        # ---- input loads ----
        xt = []
        for kt in range(KT4):
            t = big.tile([P, N], dt, name=f"xt{kt}", tag="big")
            nc.sync.dma_start(t[:], xT_d[kt])
            xt.append(t)

        def load_w(dram, nm):
            t = wp.tile([P, KT4, EMB], dt, name=nm, tag=nm)
            for kt in range(KT4):
                nc.sync.dma_start(t[:, kt], dram[kt])
            return t

        wq_s = load_w(wq_d, "wqs")
        wk_s = load_w(wk_d, "wks")
        wv_s = load_w(wv_d, "wvs")
        wo_s = load_w(wo_d, "wos")
        bq_s = pers.tile([P, KT4], dt, name="bqs")
        nc.sync.dma_start(bq_s[:], bq_d[:])
        bk_s = pers.tile([P, KT4], dt, name="bks")
        nc.sync.dma_start(bk_s[:], bk_d[:])
        bvr_s = pers.tile([P, HEADS, D], dt, name="bvrs")
        nc.sync.dma_start(bvr_s[:], bvr_d.ap().rearrange("p (h d) -> p h d", d=D))
        bor_s = pers.tile([P, EMB], dt, name="bors")
        nc.sync.dma_start(bor_s[:], bor_d[:])
        sel_s = pers.tile([HEADS, KT4, P], dt, name="sels")
        nc.sync.dma_start(sel_s[:], sel_d[:])

        # ---- persistent intermediates ----
        QT = pers.tile([P, KT4, HALF], dt, name="QT")
        KTt = pers.tile([P, KT4, N], dt, name="KTt")
        Vb = pers.tile([P, 16, HEADS, D + 8], dt, name="Vb")
        outT = pers.tile([P, KT4, HALF], dt, name="outT")
        sums = pers.tile([64 + HEADS, HALF], dt, name="sums")
        sums0 = pers.tile([HEADS, HALF], dt, name="sums0")
        rsum = pers.tile([HEADS, HALF], dt, name="rsum")

        # per-head one-hot indicator columns: PV lands head h's softmax
        # denominator on PSUM partition 64+h (32-aligned drains, distinct rows)
        nc.vector.memset(Vb[:, :, :, D:D + 8], 0.0)
        for h in range(HEADS):
            nc.vector.memset(Vb[:, :, h, D + h], 1.0)
        nc.vector.memset(sums[64:64 + HEADS, :], 0.0)

        # ---- Q^T projection ----
        for jt in range(KT4):
            for c in range(2):
                pq = ps.tile([P, 512], dt, tag="s", name=f"pq{jt}{c}")
                for kt in range(KT4):
                    nc.tensor.matmul(
                        pq[:],
                        lhsT=wq_s[:, kt, jt * P:(jt + 1) * P],
                        rhs=xt[kt][:, c * 512:(c + 1) * 512],
                        start=kt == 0,
                        stop=kt == KT4 - 1,
                    )
                nc.vector.tensor_scalar_add(
                    QT[:, jt, c * 512:(c + 1) * 512], pq[:], bq_s[:, jt:jt + 1]
                )

        # ---- K^T projection ----
        for jt in range(KT4):
            for c in range(4):
                pk = ps.tile([P, 512], dt, tag="s", name=f"pk{jt}{c}")
                for kt in range(KT4):
                    nc.tensor.matmul(
                        pk[:],
                        lhsT=wk_s[:, kt, jt * P:(jt + 1) * P],
                        rhs=xt[kt][:, c * 512:(c + 1) * 512],
                        start=kt == 0,
                        stop=kt == KT4 - 1,
                    )
                nc.vector.tensor_scalar_add(
                    KTt[:, jt, c * 512:(c + 1) * 512], pk[:], bk_s[:, jt:jt + 1]
                )

        # ---- V projection (token-major, strided into 65-wide head blocks) ----
        for t in range(16):
            pv = ps.tile([P, 512], dt, tag="s", name=f"pvv{t}")
            for kt in range(KT4):
                nc.tensor.matmul(
                    pv[:],
                    lhsT=xt[kt][:, t * P:(t + 1) * P],
                    rhs=wv_s[:, kt, :],
                    start=kt == 0,
                    stop=kt == KT4 - 1,
                )
            nc.vector.tensor_tensor(
                Vb[:, t, :, 0:D],
                pv.rearrange("p (h d) -> p h d", d=D),
                bvr_s[:],
                add,
            )

        # ---- attention: head pairs x query chunks ----
        for hp in range(4):
            hA, hB = 2 * hp, 2 * hp + 1
            jt = hp  # feature tile holding this head pair
            for c in range(2):
                pvA = ps.tile([D + 8, 512], dt, tag="pv", bufs=2, name=f"pvA{hp}{c}")
                pvB = ps.tile([D + 8, 512], dt, tag="pv", bufs=2, name=f"pvB{hp}{c}")
                for g in range(8):  # pair-groups of 2 key-tiles
                    sA = ps.tile([P, 1024], dt, tag="s", name=f"sA{hp}{c}{g}")
                    sB = ps.tile([P, 1024], dt, tag="s", name=f"sB{hp}{c}{g}")
                    for tt in range(2):
                        t = 2 * g + tt
                        # head A on PE rows 0:64, head B on rows 64:128 —
                        # concurrent via row tiling
                        nc.tensor.matmul(
                            sA[:, tt * 512:(tt + 1) * 512],
                            lhsT=KTt[0:D, jt, t * P:(t + 1) * P],
                            rhs=QT[0:D, jt, c * 512:(c + 1) * 512],
                            start=True,
                            stop=True,
                        )
                        nc.tensor.matmul(
                            sB[:, tt * 512:(tt + 1) * 512],
                            lhsT=KTt[D:P, jt, t * P:(t + 1) * P],
                            rhs=QT[D:P, jt, c * 512:(c + 1) * 512],
                            start=True,
                            stop=True,
                        )
                    ptA = big.tile([P, 1024], dt, tag="big", name=f"ptA{hp}{c}{g}")
                    ptB = big.tile([P, 1024], dt, tag="big", name=f"ptB{hp}{c}{g}")
                    nc.scalar.activation(ptA[:], sA[:], Exp, scale=SCALE)
                    if debug and hp == 0 and c == 0 and g == 0:
                        nc.sync.dma_start(dPT.ap(), ptA[:])
                    nc.scalar.activation(ptB[:], sB[:], Exp, scale=SCALE)
                    for tt in range(2):
                        t = 2 * g + tt
                        nc.tensor.matmul(
                            pvA[:],
                            lhsT=Vb[:, t, hA, :],
                            rhs=ptA[:, tt * 512:(tt + 1) * 512],
                            start=t == 0,
                            stop=t == 15,
                        )
                        nc.tensor.matmul(
                            pvB[:],
                            lhsT=Vb[:, t, hB, :],
                            rhs=ptB[:, tt * 512:(tt + 1) * 512],
                            start=t == 0,
                            stop=t == 15,
                        )
                for pv_, h in ((pvA, hA), (pvB, hB)):
                    po = (h % 2) * D
                    nc.vector.tensor_copy(
                        outT[po:po + D, h // 2, c * 512:(c + 1) * 512], pv_[0:D, :]
                    )
                    nc.vector.tensor_tensor(
                        sums[64:64 + HEADS, c * 512:(c + 1) * 512],
                        sums[64:64 + HEADS, c * 512:(c + 1) * 512],
                        pv_[D:D + 8, :],
                        add,
                    )

        if debug:
            nc.sync.dma_start(doutU.ap(), outT[:])

        # ---- normalize: outT *= broadcast(1/sums) ----
        # reciprocal_approx_fast is broken at partition base 64 — move to base 0
        nc.vector.tensor_copy(sums0[:], sums[64:64 + HEADS, :])
        nc.vector.reciprocal_approx_fast(rsum[:], sums0[:])
        for c in range(2):
            for et in range(KT4):
                pr = ps.tile([P, 512], dt, tag="s", name=f"pr{c}{et}")
                nc.tensor.matmul(
                    pr[:],
                    lhsT=sel_s[:, et, :],
                    rhs=rsum[:, c * 512:(c + 1) * 512],
                    start=True,
                    stop=True,
                )
                nc.vector.tensor_tensor(
                    outT[:, et, c * 512:(c + 1) * 512],
                    outT[:, et, c * 512:(c + 1) * 512],
                    pr[:],
                    mult,
                )

        if debug:
            nc.sync.dma_start(dQT.ap(), QT[:])
            nc.sync.dma_start(dKT.ap(), KTt[:])
            nc.sync.dma_start(dVb.ap(), Vb[:])
            nc.sync.dma_start(dsums.ap(), sums[64:64 + HEADS, :])
            nc.sync.dma_start(drsum.ap(), rsum[:])
            nc.sync.dma_start(doutT.ap(), outT[:])

        # ---- output projection ----
        for m in range(8):
            py = ps.tile([P, 512], dt, tag="s", name=f"py{m}")
            for et in range(KT4):
                nc.tensor.matmul(
                    py[:],
                    lhsT=outT[:, et, m * P:(m + 1) * P],
                    rhs=wo_s[:, et, :],
                    start=et == 0,
                    stop=et == KT4 - 1,
                )
            yt = yp.tile([P, 512], dt, tag="y", name=f"yt{m}")
            nc.vector.tensor_tensor(yt[:], py[:], bor_s[:], add)
            nc.sync.dma_start(y_d[m * P:(m + 1) * P, :], yt[:])

    nc.finalize()
    return nc


def _get_program(debug=False):
    key = ("nc", debug)
    if key not in _CACHE:
        _CACHE[key] = _build_program(debug)
    return _CACHE[key]


def _host_inputs(x, Wq, bq, Wk, bk, Wv, bv, Wo, bo):
    f32 = np.float32
    wqT = np.ascontiguousarray(np.asarray(Wq, f32).T).reshape(KT4, P, EMB)
    wkT = np.ascontiguousarray(np.asarray(Wk, f32).T).reshape(KT4, P, EMB)
    wvT = np.ascontiguousarray(np.asarray(Wv, f32).T).reshape(KT4, P, EMB)
    woT = np.ascontiguousarray(np.asarray(Wo, f32).T).reshape(KT4, P, EMB)
    bq2 = np.ascontiguousarray(np.asarray(bq, f32).reshape(KT4, P).T)
    bk2 = np.ascontiguousarray(np.asarray(bk, f32).reshape(KT4, P).T)
    bvr = np.ascontiguousarray(np.tile(np.asarray(bv, f32), (P, 1)))
    bor = np.ascontiguousarray(np.tile(np.asarray(bo, f32), (P, 1)))
    sel = np.zeros((HEADS, KT4, P), f32)
    for et in range(KT4):
        for m in range(P):
            sel[et * 2 + m // D, et, m] = 1.0

    shared = dict(wq=wqT, wk=wkT, wv=wvT, wo=woT, bq2=bq2, bk2=bk2,
                  bvr=bvr, bor=bor, sel=sel)
    x = np.asarray(x, f32)
    in_maps = []
    for c in range(NCORES):
        b, hf = c // 2, c % 2
        xb = x[b]
        # queries first; key order is irrelevant as long as K and V agree
        xr = np.concatenate(
            [xb[hf * HALF:(hf + 1) * HALF], xb[(1 - hf) * HALF:(2 - hf) * HALF]], 0
        )
        xT = np.ascontiguousarray(xr.T).reshape(KT4, P, N)
        in_maps.append(dict(shared, xT=xT))
    return in_maps


def kernel(x, Wq, bq, Wk, bk, Wv, bv, Wo, bo, _trace=False, _trace_cores=None,
           _debug=False):
    from concourse.bass_utils import run_bass_kernel_spmd

    nc = _get_program(_debug)
    in_maps = _host_inputs(x, Wq, bq, Wk, bk, Wv, bv, Wo, bo)
    res = run_bass_kernel_spmd(
        nc, in_maps, list(range(NCORES)), trace=_trace,
        trace_cores=_trace_cores,
    )
    out = np.empty((B, N, EMB), np.float32)
    for c in range(NCORES):
        b, hf = c // 2, c % 2
        out[b, hf * HALF:(hf + 1) * HALF] = res.results[c]["y"]
    if _trace:
        _CACHE["last_results"] = res
    return out
